# revision 11
# baseline (speedup 1.0000x reference)
"""BEVSDTransformerDecoder — Trainium2 Bass kernel (8-core SPMD).

Algorithm: multi-camera deformable attention, computed exactly (no gathers):
for each (camera, level) the sparse bilinear-sampling contraction is written
as  out^T += F^T(HW,C)^T-matmul with a dense weight matrix A(Q, HW) built on
DVE from triangle kernels: relu(1 - |iota - px|) is exactly the bilinear
weight profile of a sample at pixel coordinate px (zero padding automatic).

Sharding (uniform SPMD program): the 6 cams x 32 sample-slots = 192 global
slots are split into 24 single-camera groups of 8 slots; each of the 8 cores
processes 3 groups (24 slots) over all 4 levels.  Per-core weight-column
permutations (host-side layout prep of W_off/W_attn) select each core's
slots, so every core runs the identical program.  Host sums the per-core
partial outputs (the all-reduce of the masked scatter-add over cameras).
"""

import os
import sys
import numpy as np
from contextlib import ExitStack

sys.path.insert(0, "/opt/trn_rl_repo")

import concourse.bass as bass
import concourse.bacc as bacc
import concourse.tile as tile
from concourse import mybir
from concourse.bass_utils import run_bass_kernel_spmd

F32 = mybir.dt.float32
BF16 = mybir.dt.bfloat16
ALU = mybir.AluOpType
ACTF = mybir.ActivationFunctionType

NH, NL, NPIL, NPT = 4, 4, 4, 2
PP = NPIL * NPT
Q, C, NCAM = 1024, 256, 6
IMG_H, IMG_W, EPS = 256.0, 704.0, 1e-5
PC_LOW = np.array([-51.2, -51.2, -5.0], np.float32)
PC_SPAN = np.array([102.4, 102.4, 8.0], np.float32)
FEATS_HW = [(32, 88), (16, 44), (8, 22), (4, 11)]
NSLOT = 24          # slots per core
NGRP = 3            # single-camera groups of 8 slots per core
GSL = 8             # slots per group
NQT = 8             # q tiles of 128
MAXW, MAXH = 88, 32


def _slot_decode(gid):
    n, s = gid // 32, gid % 32
    return n, s // 8, (s % 8) // 2, s % 2      # cam, head, pillar, point


_MAKESPAN_NS = None


def _build_program():
    global _MAKESPAN_NS
    import concourse.bass_interp as _bi
    _orig_sim = _bi.CoreSim.simulate
    _times = []

    def _patched(self, *a, **k):
        r = _orig_sim(self, *a, **k)
        try:
            _times.append(int(self.time))
        except Exception:
            pass
        return r

    _bi.CoreSim.simulate = _patched
    try:
        nc = _build_program_inner()
    finally:
        _bi.CoreSim.simulate = _orig_sim
    if _times:
        _MAKESPAN_NS = max(_times)
    return nc


def _build_program_inner():
    nc = bacc.Bacc("TRN2", target_bir_lowering=False, debug=False, num_devices=8)
    dp = nc.declare_dram_parameter
    t_qT = dp("qT", [C, Q], F32, isOutput=False)
    t_qpT = dp("qposT", [C, Q], F32, isOutput=False)
    t_qresT = dp("qresT", [C, Q], F32, isOutput=False)
    t_Woff = dp("Woff", [C, 256], F32, isOutput=False)
    t_boff = dp("boff", [1, 256], F32, isOutput=False)
    t_Wattn = dp("Wattn", [C, 128], F32, isOutput=False)
    t_battn = dp("battn", [1, 128], F32, isOutput=False)
    t_Wout = dp("Wout", [C, C], F32, isOutput=False)
    t_bout = dp("boutC", [C, 1], F32, isOutput=False)
    t_ref = dp("refS", [Q, NSLOT * 12], F32, isOutput=False)
    t_Lexp = dp("Lexp", [128, 12 * NSLOT * 4], F32, isOutput=False)
    t_iox = dp("iox", [128, MAXW], F32, isOutput=False)
    t_ioy = dp("ioy", [128, MAXH], F32, isOutput=False)
    t_id = dp("ident", [128, 128], BF16, isOutput=False)
    t_ones = dp("ones", [1, Q], F32, isOutput=False)
    t_F = {}
    for g in range(NGRP):
        for l, (H, W) in enumerate(FEATS_HW):
            t_F[(g, l)] = dp(f"F{g}{l}", [H * W, C], BF16, isOutput=False)
    t_out = dp("outT", [C, Q], F32, isOutput=True)

    with tile.TileContext(nc) as tc, ExitStack() as ctx:
        cpool = ctx.enter_context(tc.tile_pool(name="consts", bufs=1))
        ppool = ctx.enter_context(tc.tile_pool(name="proj", bufs=2))
        apool = ctx.enter_context(tc.tile_pool(name="A", bufs=5))
        tpool = ctx.enter_context(tc.tile_pool(name="tmp", bufs=2))
        xpool = ctx.enter_context(tc.tile_pool(name="tri", bufs=3))
        fpool = ctx.enter_context(tc.tile_pool(name="feat", bufs=2))
        atp = ctx.enter_context(tc.tile_pool(name="AT", bufs=2))
        pspool = ctx.enter_context(tc.tile_pool(name="ps", bufs=2, space="PSUM"))
        accps = ctx.enter_context(tc.tile_pool(name="accps", bufs=1, space="PSUM"))

        # ---- load constants ----
        def load(shape, src, name):
            t = cpool.tile(shape, F32, tag=name, name=name)
            nc.sync.dma_start(t[:], src)
            return t

        woff = [load([128, 256], t_Woff[k * 128:(k + 1) * 128, :], f"woff{k}") for k in range(2)]
        wattn = [load([128, 128], t_Wattn[k * 128:(k + 1) * 128, :], f"wattn{k}") for k in range(2)]
        wout = [load([128, 256], t_Wout[k * 128:(k + 1) * 128, :], f"wout{k}") for k in range(2)]
        boutc = [load([128, 1], t_bout[k * 128:(k + 1) * 128, :], f"bout{k}") for k in range(2)]
        boff = load([1, 256], t_boff[:, :], "boff")
        battn = load([1, 128], t_battn[:, :], "battn")
        lexp = load([128, 12 * 96], t_Lexp[:, :], "lexp")
        iox = load([128, MAXW], t_iox[:, :], "iox")
        ioy = load([128, MAXH], t_ioy[:, :], "ioy")
        ident = cpool.tile([128, 128], BF16, tag="ident", name="ident")
        nc.sync.dma_start(ident[:], t_id[:, :])
        ones = load([1, Q], t_ones[:, :], "ones")

        # qp^T = (query + query_pos)^T   [2 x (128, 1024)]
        qpT = []
        for k in range(2):
            a = ppool.tile([128, Q], F32, tag="qld", bufs=1)
            b = ppool.tile([128, Q], F32, tag="qld2", bufs=1)
            nc.sync.dma_start(a[:], t_qT[k * 128:(k + 1) * 128, :])
            nc.sync.dma_start(b[:], t_qpT[k * 128:(k + 1) * 128, :])
            s = cpool.tile([128, Q], F32, tag=f"qpT{k}")
            nc.vector.tensor_add(s[:], a[:], b[:])
            qpT.append(s)

        # per-q-tile persistent small tensors
        pxn = [cpool.tile([128, 96], F32, tag=f"pxn{m}", name=f"pxn{m}") for m in range(NQT)]
        pyn = [cpool.tile([128, 96], F32, tag=f"pyn{m}", name=f"pyn{m}") for m in range(NQT)]
        aef = [cpool.tile([128, 96], F32, tag=f"aef{m}", name=f"aef{m}") for m in range(NQT)]

        # ---- per q-tile: linear layers + projection ----
        for m in range(NQT):
            qsl = slice(m * 128, (m + 1) * 128)
            offp = pspool.tile([128, 256], F32, tag="scps", name="offp", bufs=2, padded_shape=[128, 512])
            for k in range(2):
                nc.tensor.matmul(offp[:], qpT[k][:, qsl], woff[k][:],
                                 start=(k == 0), stop=False)
            nc.tensor.matmul(offp[:], ones[:, qsl], boff[:],
                             start=False, stop=True)
            attp = pspool.tile([128, 128], F32, tag="scps", name="attp", bufs=2, padded_shape=[128, 512])
            for k in range(2):
                nc.tensor.matmul(attp[:], qpT[k][:, qsl], wattn[k][:],
                                 start=(k == 0), stop=False)
            nc.tensor.matmul(attp[:], ones[:, qsl], battn[:],
                             start=False, stop=True)
            off_sb = ppool.tile([128, 256], F32, tag="offsb")
            nc.scalar.copy(off_sb[:], offp[:])
            attnw = ppool.tile([128, 128], F32, tag="attnw")
            nc.scalar.activation(attnw[:], attp[:], ACTF.Sigmoid)

            refm = ppool.tile([128, NSLOT * 12], F32, tag="refm")
            nc.sync.dma_start(refm[:], t_ref[qsl, :])
            r3 = refm[:].rearrange("p (s c) -> p s c", c=3)
            X, Y, Z = r3[:, :, 0], r3[:, :, 1], r3[:, :, 2]

            def LP(i):
                return lexp[:, i * 96:(i + 1) * 96]

            uvd = []
            for comp in range(3):
                acc = ppool.tile([128, 96], F32, tag=f"uvd{comp}", name=f"uvd{comp}", bufs=1)
                nc.vector.tensor_mul(acc[:], X, LP(comp * 4 + 0))
                tmp2 = ppool.tile([128, 96], F32, tag="projtmp")
                nc.vector.tensor_mul(tmp2[:], Y, LP(comp * 4 + 1))
                nc.vector.tensor_add(acc[:], acc[:], tmp2[:])
                nc.vector.tensor_mul(tmp2[:], Z, LP(comp * 4 + 2))
                nc.vector.tensor_add(acc[:], acc[:], tmp2[:])
                nc.vector.tensor_add(acc[:], acc[:], LP(comp * 4 + 3))
                uvd.append(acc)
            u, v, d = uvd
            dcl = ppool.tile([128, 96], F32, tag="dcl")
            nc.vector.tensor_scalar(dcl[:], d[:], float(EPS), None, ALU.max)
            val = ppool.tile([128, 96], F32, tag="val")
            nc.vector.tensor_scalar(val[:], d[:], float(EPS), None, ALU.is_gt)
            tmpv = ppool.tile([128, 96], F32, tag="tmpv")
            nc.vector.tensor_scalar(tmpv[:], u[:], 0.0, None, ALU.is_gt)
            nc.vector.tensor_mul(val[:], val[:], tmpv[:])
            nc.vector.tensor_scalar(tmpv[:], v[:], 0.0, None, ALU.is_gt)
            nc.vector.tensor_mul(val[:], val[:], tmpv[:])
            lim = ppool.tile([128, 96], F32, tag="lim")
            nc.vector.tensor_scalar(lim[:], dcl[:], float(IMG_W), None, ALU.mult)
            nc.vector.tensor_tensor(tmpv[:], u[:], lim[:], ALU.is_lt)
            nc.vector.tensor_mul(val[:], val[:], tmpv[:])
            nc.vector.tensor_scalar(lim[:], dcl[:], float(IMG_H), None, ALU.mult)
            nc.vector.tensor_tensor(tmpv[:], v[:], lim[:], ALU.is_lt)
            nc.vector.tensor_mul(val[:], val[:], tmpv[:])
            qmask = ppool.tile([128, 24], F32, tag="qmask")
            nc.vector.tensor_reduce(qmask[:], val[:].rearrange("p (s r) -> p s r", r=4),
                                    mybir.AxisListType.X, ALU.max)

            # own-pillar grid coords
            rec = ppool.tile([128, 24], F32, tag="rec")
            d4 = dcl[:].rearrange("p (s r) -> p s r", r=4)
            nc.vector.reciprocal(rec[:], d4[:, :, 0])
            gx = ppool.tile([128, 24], F32, tag="gx")
            u4 = u[:].rearrange("p (s r) -> p s r", r=4)
            nc.vector.tensor_mul(gx[:], u4[:, :, 0], rec[:])
            nc.vector.tensor_scalar(gx[:], gx[:], float(2.0 / IMG_W), -1.0, ALU.mult, ALU.add)
            gy = ppool.tile([128, 24], F32, tag="gy")
            v4 = v[:].rearrange("p (s r) -> p s r", r=4)
            nc.vector.tensor_mul(gy[:], v4[:, :, 0], rec[:])
            nc.vector.tensor_scalar(gy[:], gy[:], float(2.0 / IMG_H), -1.0, ALU.mult, ALU.add)

            offr = off_sb[:].rearrange("p (j r) -> p j r", r=8)
            attr = attnw[:].rearrange("p (j r) -> p j r", r=4)
            for l, (H, W) in enumerate(FEATS_HW):
                lsl = slice(l * 24, (l + 1) * 24)
                sx = ppool.tile([128, 24], F32, tag="sx")
                nc.vector.tensor_add(sx[:], gx[:], offr[:, :24, 2 * l + 0])
                nc.vector.tensor_scalar(pxn[m][:, lsl], sx[:], float(-W / 2.0),
                                        float(0.5 - W / 2.0), ALU.mult, ALU.add)
                nc.vector.tensor_add(sx[:], gy[:], offr[:, :24, 2 * l + 1])
                nc.vector.tensor_scalar(pyn[m][:, lsl], sx[:], float(-H / 2.0),
                                        float(0.5 - H / 2.0), ALU.mult, ALU.add)
                nc.vector.tensor_tensor(aef[m][:, lsl], attr[:, :24, l], qmask[:], ALU.mult)

        # ---- main build + matmul ----
        accT = [cpool.tile([128, Q], F32, tag=f"accT{k}", name=f"accT{k}") for k in range(2)]
        acc_ps = [[accps.tile([128, 512], F32, tag=f"acc{cc}h{h}", name=f"acc{cc}h{h}") for h in range(2)]
                  for cc in range(2)]

        for g in range(NGRP):
            for l, (H, W) in enumerate(FEATS_HW):
                HW = H * W
                KT = (HW + 127) // 128
                fsb = fpool.tile([128, KT * 256], BF16, tag="fsb")
                for kt in range(KT):
                    ksz = min(128, HW - kt * 128)
                    nc.sync.dma_start(fsb[:ksz, kt * 256:(kt + 1) * 256],
                                      t_F[(g, l)][kt * 128:kt * 128 + ksz, :])
                first_gl = (g == 0 and l == 0)
                last_gl = (g == NGRP - 1 and l == NL - 1)
                for half in range(2):
                    Ats = []
                    for mm in range(4):
                        m = half * 4 + mm
                        A = apool.tile([128, HW], BF16, tag="A")
                        Ats.append(A)
                        base = l * 24 + g * 8
                        dx = xpool.tile([128, GSL * MAXW], F32, tag="dx")
                        dxv = dx[:, :GSL * W].rearrange("p (s w) -> p s w", w=W)
                        nc.vector.tensor_tensor(
                            dxv,
                            iox[:, :W].unsqueeze(1).broadcast_to([128, GSL, W]),
                            pxn[m][:, base:base + GSL].unsqueeze(2).broadcast_to([128, GSL, W]),
                            ALU.add)
                        tx = xpool.tile([128, GSL * MAXW], BF16, tag="tx")
                        nc.scalar.activation(dx[:, :GSL * W], dx[:, :GSL * W], ACTF.Abs)
                        nc.scalar.activation(tx[:, :GSL * W], dx[:, :GSL * W], ACTF.Relu,
                                             bias=1.0, scale=-1.0)
                        dy = xpool.tile([128, GSL * MAXH], F32, tag="dy")
                        dyv = dy[:, :GSL * H].rearrange("p (s h) -> p s h", h=H)
                        nc.vector.tensor_tensor(
                            dyv,
                            ioy[:, :H].unsqueeze(1).broadcast_to([128, GSL, H]),
                            pyn[m][:, base:base + GSL].unsqueeze(2).broadcast_to([128, GSL, H]),
                            ALU.add)
                        ty = xpool.tile([128, GSL * MAXH], BF16, tag="ty")
                        nc.scalar.activation(dy[:, :GSL * H], dy[:, :GSL * H], ACTF.Abs)
                        nc.scalar.activation(ty[:, :GSL * H], dy[:, :GSL * H], ACTF.Relu,
                                             bias=1.0, scale=-1.0)
                        txv = tx[:, :GSL * W].rearrange("p (s w) -> p s w", w=W)
                        tyv = ty[:, :GSL * H].rearrange("p (s h) -> p s h", h=H)
                        for js in range(GSL):
                            tmp = tpool.tile([128, HW], BF16, tag="tmp", bufs=1)
                            tmpv = tmp[:].rearrange("p (h w) -> p h w", w=W)
                            nc.vector.tensor_tensor(
                                tmpv,
                                txv[:, js].unsqueeze(1).broadcast_to([128, H, W]),
                                tyv[:, js].unsqueeze(2).broadcast_to([128, H, W]),
                                ALU.mult)
                            aesc = aef[m][:, l * 24 + g * 8 + js:l * 24 + g * 8 + js + 1]
                            if js == 0:
                                nc.vector.tensor_scalar(A[:], tmp[:], aesc, None, ALU.mult)
                            else:
                                nc.vector.scalar_tensor_tensor(
                                    A[:], tmp[:], aesc, A[:], ALU.mult, ALU.add)
                    for kt in range(KT):
                        ksz = min(128, HW - kt * 128)
                        AT = atp.tile([128, 512], BF16, tag="AT")
                        for mm in range(4):
                            tp = pspool.tile([128, 128], BF16, tag="tp", bufs=2)
                            nc.tensor.transpose(tp[:ksz, :],
                                                Ats[mm][:, kt * 128:kt * 128 + ksz],
                                                ident[:])
                            nc.scalar.copy(AT[:ksz, mm * 128:(mm + 1) * 128], tp[:ksz, :])
                        for cc in range(2):
                            nc.tensor.matmul(
                                acc_ps[cc][half][:],
                                fsb[:ksz, kt * 256 + cc * 128:kt * 256 + (cc + 1) * 128],
                                AT[:ksz, :],
                                start=(first_gl and kt == 0),
                                stop=(last_gl and kt == KT - 1))

        for cc in range(2):
            for half in range(2):
                nc.vector.tensor_copy(accT[cc][:, half * 512:(half + 1) * 512],
                                      acc_ps[cc][half][:])

        # ---- final linear + bias + residual ----
        qres = [ppool.tile([128, Q], F32, tag=f"qres{k}", name=f"qres{k}", bufs=1) for k in range(2)]
        for k in range(2):
            nc.sync.dma_start(qres[k][:], t_qresT[k * 128:(k + 1) * 128, :])
        for cc in range(2):
            for qc in range(2):
                op = pspool.tile([128, 512], F32, tag="scps", name="outp", bufs=2)
                for k in range(2):
                    nc.tensor.matmul(op[:],
                                     wout[k][:, cc * 128:(cc + 1) * 128],
                                     accT[k][:, qc * 512:(qc + 1) * 512],
                                     start=(k == 0), stop=(k == 1))
                ob = tpool.tile([128, 512], F32, tag="ob")
                nc.vector.scalar_tensor_tensor(
                    ob[:], op[:], boutc[cc][:, 0:1],
                    qres[cc][:, qc * 512:(qc + 1) * 512], ALU.add, ALU.add)
                nc.sync.dma_start(t_out[cc * 128:(cc + 1) * 128, qc * 512:(qc + 1) * 512],
                                  ob[:])
    nc.compile()
    return nc


def _prep_inputs(inp):
    query = np.ascontiguousarray(inp["query"][0], np.float32)
    qpos = np.ascontiguousarray(inp["query_pos"][0], np.float32)
    ref_q = np.transpose(inp["reference_points"], (0, 2, 3, 1, 4)).reshape(Q, NPIL, 3)
    ref_scaled = (ref_q * PC_SPAN + PC_LOW).astype(np.float32)
    L_all = np.asarray(inp["lidar2img"][0], np.float32)
    qT = np.ascontiguousarray(query.T)
    qposT = np.ascontiguousarray(qpos.T)
    iox = np.tile(np.arange(MAXW, dtype=np.float32), (128, 1))
    ioy = np.tile(np.arange(MAXH, dtype=np.float32), (128, 1))
    ident = np.eye(128, dtype=np.float32)
    ones = np.ones((1, Q), np.float32)
    Wout = np.ascontiguousarray(inp["W_out"], np.float32)
    boutC = np.ascontiguousarray(inp["b_out"].reshape(C, 1), np.float32)
    zeros_cq = np.zeros((C, Q), np.float32)

    in_maps = []
    for core in range(8):
        slots = [_slot_decode(core * NSLOT + j) for j in range(NSLOT)]
        W_off_c = np.zeros((C, 256), np.float32)
        b_off_c = np.zeros((1, 256), np.float32)
        W_attn_c = np.zeros((C, 128), np.float32)
        b_attn_c = np.zeros((1, 128), np.float32)
        refc = np.zeros((Q, NSLOT, 4, 3), np.float32)
        Lexp = np.zeros((12, NSLOT, 4), np.float32)
        for j, (n, h, pil, t) in enumerate(slots):
            for l in range(NL):
                for xy in range(2):
                    src = (((h * NL + l) * NPIL + pil) * NPT + t) * 2 + xy
                    dst = (j * NL + l) * 2 + xy
                    W_off_c[:, dst] = inp["W_off"][:, src]
                    b_off_c[0, dst] = inp["b_off"][src]
                srca = (h * PP + pil * NPT + t) * NL + l
                W_attn_c[:, j * NL + l] = inp["W_attn"][:, srca]
                b_attn_c[0, j * NL + l] = inp["b_attn"][srca]
            order = [pil] + [p for p in range(4) if p != pil]
            refc[:, j] = ref_scaled[:, order]
            for i in range(3):
                for k in range(4):
                    Lexp[i * 4 + k, j, :] = L_all[n][i, k]
        m = {
            "qT": qT, "qposT": qposT,
            "qresT": qT if core == 0 else zeros_cq,
            "Woff": W_off_c, "boff": b_off_c,
            "Wattn": W_attn_c, "battn": b_attn_c,
            "Wout": Wout,
            "boutC": boutC if core == 0 else np.zeros((C, 1), np.float32),
            "refS": np.ascontiguousarray(refc.reshape(Q, NSLOT * 12)),
            "Lexp": np.ascontiguousarray(
                np.tile(Lexp.reshape(1, 12 * NSLOT * 4), (128, 1))),
            "iox": iox, "ioy": ioy, "ones": ones,
            "ident": np.eye(128, dtype=np.float32).astype(__import__("ml_dtypes").bfloat16),
        }
        for g in range(NGRP):
            cam = (core * NGRP + g) // 4
            for l, (H, W) in enumerate(FEATS_HW):
                F = np.asarray(inp[f"feat{l}"][0, cam], np.float32).reshape(C, H * W)
                import ml_dtypes
                m[f"F{g}{l}"] = np.ascontiguousarray(F.T).astype(ml_dtypes.bfloat16)
        in_maps.append(m)
    return in_maps


_NC = None


def kernel(**inputs):
    global _NC
    inp = {k: np.asarray(v) for k, v in inputs.items()}
    if _NC is None:
        _NC = _build_program()
    in_maps = _prep_inputs(inp)
    res = run_bass_kernel_spmd(_NC, in_maps, core_ids=list(range(8)))
    out = np.zeros((C, Q), np.float32)
    for r in res.results:
        out += np.asarray(r["outT"], np.float32)
    return np.ascontiguousarray(out.T).reshape(1, Q, C)


# revision 12
# speedup vs baseline: 2227.4096x; 2227.4096x over previous
"""BEVSDTransformerDecoder — Trainium2 Bass kernel (8-core SPMD).

Algorithm: multi-camera deformable attention, computed exactly (no gathers):
for each (camera, level) the sparse bilinear-sampling contraction is written
as  out^T += F^T(HW,C)^T-matmul with a dense weight matrix A(Q, HW) built on
DVE from triangle kernels: relu(1 - |iota - px|) is exactly the bilinear
weight profile of a sample at pixel coordinate px (zero padding automatic).

Sharding (uniform SPMD program): the 6 cams x 32 sample-slots = 192 global
slots are split into 24 single-camera groups of 8 slots; each of the 8 cores
processes 3 groups (24 slots) over all 4 levels.  Per-core weight-column
permutations (host-side layout prep of W_off/W_attn) select each core's
slots, so every core runs the identical program.  Host sums the per-core
partial outputs (the all-reduce of the masked scatter-add over cameras).
"""

import os
import sys
import numpy as np
from contextlib import ExitStack

sys.path.insert(0, "/opt/trn_rl_repo")

import concourse.bass as bass
import concourse.bacc as bacc
import concourse.tile as tile
from concourse import mybir
from concourse.bass_utils import run_bass_kernel_spmd

F32 = mybir.dt.float32
BF16 = mybir.dt.bfloat16
ALU = mybir.AluOpType
ACTF = mybir.ActivationFunctionType

NH, NL, NPIL, NPT = 4, 4, 4, 2
PP = NPIL * NPT
Q, C, NCAM = 1024, 256, 6
IMG_H, IMG_W, EPS = 256.0, 704.0, 1e-5
PC_LOW = np.array([-51.2, -51.2, -5.0], np.float32)
PC_SPAN = np.array([102.4, 102.4, 8.0], np.float32)
FEATS_HW = [(32, 88), (16, 44), (8, 22), (4, 11)]
NSLOT = 24          # slots per core
NGRP = 3            # single-camera groups of 8 slots per core
GSL = 8             # slots per group
NQT = 8             # q tiles of 128
MAXW, MAXH = 88, 32


def _slot_decode(gid):
    n, s = gid // 32, gid % 32
    return n, s // 8, (s % 8) // 2, s % 2      # cam, head, pillar, point


_MAKESPAN_NS = None


def _build_program():
    global _MAKESPAN_NS
    import concourse.bass_interp as _bi
    _orig_sim = _bi.CoreSim.simulate
    _times = []

    def _patched(self, *a, **k):
        r = _orig_sim(self, *a, **k)
        try:
            _times.append(int(self.time))
        except Exception:
            pass
        return r

    _bi.CoreSim.simulate = _patched
    try:
        nc = _build_program_inner()
    finally:
        _bi.CoreSim.simulate = _orig_sim
    if _times:
        _MAKESPAN_NS = max(_times)
    return nc


def _build_program_inner():
    nc = bacc.Bacc("TRN2", target_bir_lowering=False, debug=False, num_devices=8)
    dp = nc.declare_dram_parameter
    t_qT = dp("qT", [C, Q], F32, isOutput=False)
    t_qpT = dp("qposT", [C, Q], F32, isOutput=False)
    t_qresT = dp("qresT", [C, Q], F32, isOutput=False)
    t_Woff = dp("Woff", [C, 256], F32, isOutput=False)
    t_boff = dp("boff", [1, 256], F32, isOutput=False)
    t_Wattn = dp("Wattn", [C, 128], F32, isOutput=False)
    t_battn = dp("battn", [1, 128], F32, isOutput=False)
    t_Wout = dp("Wout", [C, C], F32, isOutput=False)
    t_bout = dp("boutC", [C, 1], F32, isOutput=False)
    t_ref = dp("refS", [Q, NSLOT * 12], F32, isOutput=False)
    t_Lexp = dp("Lexp", [128, 12 * NSLOT * 4], F32, isOutput=False)
    t_iox = dp("iox", [128, MAXW], F32, isOutput=False)
    t_ioy = dp("ioy", [128, MAXH], F32, isOutput=False)
    t_id = dp("ident", [128, 128], BF16, isOutput=False)
    t_ones = dp("ones", [1, Q], F32, isOutput=False)
    t_F = {}
    for g in range(NGRP):
        for l, (H, W) in enumerate(FEATS_HW):
            t_F[(g, l)] = dp(f"F{g}{l}", [H * W, C], BF16, isOutput=False)
    t_out = dp("outT", [C, Q], F32, isOutput=True)

    with tile.TileContext(nc) as tc, ExitStack() as ctx:
        cpool = ctx.enter_context(tc.tile_pool(name="consts", bufs=1))
        ppool = ctx.enter_context(tc.tile_pool(name="proj", bufs=2))
        apool = ctx.enter_context(tc.tile_pool(name="A", bufs=5))
        tpool = ctx.enter_context(tc.tile_pool(name="tmp", bufs=2))
        xpool = ctx.enter_context(tc.tile_pool(name="tri", bufs=3))
        fpool = ctx.enter_context(tc.tile_pool(name="feat", bufs=2))
        atp = ctx.enter_context(tc.tile_pool(name="AT", bufs=2))
        pspool = ctx.enter_context(tc.tile_pool(name="ps", bufs=2, space="PSUM"))
        accps = ctx.enter_context(tc.tile_pool(name="accps", bufs=1, space="PSUM"))

        # ---- load constants ----
        def load(shape, src, name):
            t = cpool.tile(shape, F32, tag=name, name=name)
            nc.sync.dma_start(t[:], src)
            return t

        woff = [load([128, 256], t_Woff[k * 128:(k + 1) * 128, :], f"woff{k}") for k in range(2)]
        wattn = [load([128, 128], t_Wattn[k * 128:(k + 1) * 128, :], f"wattn{k}") for k in range(2)]
        wout = [load([128, 256], t_Wout[k * 128:(k + 1) * 128, :], f"wout{k}") for k in range(2)]
        boutc = [load([128, 1], t_bout[k * 128:(k + 1) * 128, :], f"bout{k}") for k in range(2)]
        boff = load([1, 256], t_boff[:, :], "boff")
        battn = load([1, 128], t_battn[:, :], "battn")
        lexp = load([128, 12 * 96], t_Lexp[:, :], "lexp")
        iox = load([128, MAXW], t_iox[:, :], "iox")
        ioy = load([128, MAXH], t_ioy[:, :], "ioy")
        ident = cpool.tile([128, 128], BF16, tag="ident", name="ident")
        nc.sync.dma_start(ident[:], t_id[:, :])
        ones = load([1, Q], t_ones[:, :], "ones")

        # qp^T = (query + query_pos)^T   [2 x (128, 1024)]
        qpT = []
        for k in range(2):
            a = ppool.tile([128, Q], F32, tag="qld", bufs=1)
            b = ppool.tile([128, Q], F32, tag="qld2", bufs=1)
            nc.sync.dma_start(a[:], t_qT[k * 128:(k + 1) * 128, :])
            nc.sync.dma_start(b[:], t_qpT[k * 128:(k + 1) * 128, :])
            s = cpool.tile([128, Q], F32, tag=f"qpT{k}")
            nc.vector.tensor_add(s[:], a[:], b[:])
            qpT.append(s)

        # per-q-tile persistent small tensors
        pxn = [cpool.tile([128, 96], F32, tag=f"pxn{m}", name=f"pxn{m}") for m in range(NQT)]
        pyn = [cpool.tile([128, 96], F32, tag=f"pyn{m}", name=f"pyn{m}") for m in range(NQT)]
        aef = [cpool.tile([128, 96], F32, tag=f"aef{m}", name=f"aef{m}") for m in range(NQT)]

        # ---- per q-tile: linear layers + projection ----
        for m in range(NQT):
            qsl = slice(m * 128, (m + 1) * 128)
            offp = pspool.tile([128, 256], F32, tag="scps", name="offp", bufs=2, padded_shape=[128, 512])
            for k in range(2):
                nc.tensor.matmul(offp[:], qpT[k][:, qsl], woff[k][:],
                                 start=(k == 0), stop=False)
            nc.tensor.matmul(offp[:], ones[:, qsl], boff[:],
                             start=False, stop=True)
            attp = pspool.tile([128, 128], F32, tag="scps", name="attp", bufs=2, padded_shape=[128, 512])
            for k in range(2):
                nc.tensor.matmul(attp[:], qpT[k][:, qsl], wattn[k][:],
                                 start=(k == 0), stop=False)
            nc.tensor.matmul(attp[:], ones[:, qsl], battn[:],
                             start=False, stop=True)
            off_sb = ppool.tile([128, 256], F32, tag="offsb")
            nc.scalar.copy(off_sb[:], offp[:])
            attnw = ppool.tile([128, 128], F32, tag="attnw")
            nc.scalar.activation(attnw[:], attp[:], ACTF.Sigmoid)

            refm = ppool.tile([128, NSLOT * 12], F32, tag="refm")
            nc.sync.dma_start(refm[:], t_ref[qsl, :])
            r3 = refm[:].rearrange("p (s c) -> p s c", c=3)
            X, Y, Z = r3[:, :, 0], r3[:, :, 1], r3[:, :, 2]

            def LP(i):
                return lexp[:, i * 96:(i + 1) * 96]

            uvd = []
            for comp in range(3):
                acc = ppool.tile([128, 96], F32, tag=f"uvd{comp}", name=f"uvd{comp}", bufs=1)
                nc.vector.tensor_mul(acc[:], X, LP(comp * 4 + 0))
                tmp2 = ppool.tile([128, 96], F32, tag="projtmp")
                nc.vector.tensor_mul(tmp2[:], Y, LP(comp * 4 + 1))
                nc.vector.tensor_add(acc[:], acc[:], tmp2[:])
                nc.vector.tensor_mul(tmp2[:], Z, LP(comp * 4 + 2))
                nc.vector.tensor_add(acc[:], acc[:], tmp2[:])
                nc.vector.tensor_add(acc[:], acc[:], LP(comp * 4 + 3))
                uvd.append(acc)
            u, v, d = uvd
            dcl = ppool.tile([128, 96], F32, tag="dcl")
            nc.vector.tensor_scalar(dcl[:], d[:], float(EPS), None, ALU.max)
            val = ppool.tile([128, 96], F32, tag="val")
            nc.vector.tensor_scalar(val[:], d[:], float(EPS), None, ALU.is_gt)
            tmpv = ppool.tile([128, 96], F32, tag="tmpv")
            nc.vector.tensor_scalar(tmpv[:], u[:], 0.0, None, ALU.is_gt)
            nc.vector.tensor_mul(val[:], val[:], tmpv[:])
            nc.vector.tensor_scalar(tmpv[:], v[:], 0.0, None, ALU.is_gt)
            nc.vector.tensor_mul(val[:], val[:], tmpv[:])
            lim = ppool.tile([128, 96], F32, tag="lim")
            nc.vector.tensor_scalar(lim[:], dcl[:], float(IMG_W), None, ALU.mult)
            nc.vector.tensor_tensor(tmpv[:], u[:], lim[:], ALU.is_lt)
            nc.vector.tensor_mul(val[:], val[:], tmpv[:])
            nc.vector.tensor_scalar(lim[:], dcl[:], float(IMG_H), None, ALU.mult)
            nc.vector.tensor_tensor(tmpv[:], v[:], lim[:], ALU.is_lt)
            nc.vector.tensor_mul(val[:], val[:], tmpv[:])
            qmask = ppool.tile([128, 24], F32, tag="qmask")
            nc.vector.tensor_reduce(qmask[:], val[:].rearrange("p (s r) -> p s r", r=4),
                                    mybir.AxisListType.X, ALU.max)

            # own-pillar grid coords
            rec = ppool.tile([128, 24], F32, tag="rec")
            d4 = dcl[:].rearrange("p (s r) -> p s r", r=4)
            nc.vector.reciprocal(rec[:], d4[:, :, 0])
            gx = ppool.tile([128, 24], F32, tag="gx")
            u4 = u[:].rearrange("p (s r) -> p s r", r=4)
            nc.vector.tensor_mul(gx[:], u4[:, :, 0], rec[:])
            nc.vector.tensor_scalar(gx[:], gx[:], float(2.0 / IMG_W), -1.0, ALU.mult, ALU.add)
            gy = ppool.tile([128, 24], F32, tag="gy")
            v4 = v[:].rearrange("p (s r) -> p s r", r=4)
            nc.vector.tensor_mul(gy[:], v4[:, :, 0], rec[:])
            nc.vector.tensor_scalar(gy[:], gy[:], float(2.0 / IMG_H), -1.0, ALU.mult, ALU.add)

            offr = off_sb[:].rearrange("p (j r) -> p j r", r=8)
            attr = attnw[:].rearrange("p (j r) -> p j r", r=4)
            for l, (H, W) in enumerate(FEATS_HW):
                lsl = slice(l * 24, (l + 1) * 24)
                sx = ppool.tile([128, 24], F32, tag="sx")
                nc.vector.tensor_add(sx[:], gx[:], offr[:, :24, 2 * l + 0])
                nc.vector.tensor_scalar(pxn[m][:, lsl], sx[:], float(-W / 2.0),
                                        float(0.5 - W / 2.0), ALU.mult, ALU.add)
                nc.vector.tensor_add(sx[:], gy[:], offr[:, :24, 2 * l + 1])
                nc.vector.tensor_scalar(pyn[m][:, lsl], sx[:], float(-H / 2.0),
                                        float(0.5 - H / 2.0), ALU.mult, ALU.add)
                nc.vector.tensor_tensor(aef[m][:, lsl], attr[:, :24, l], qmask[:], ALU.mult)

        # ---- main build + matmul ----
        accT = [cpool.tile([128, Q], F32, tag=f"accT{k}", name=f"accT{k}") for k in range(2)]
        acc_ps = [[accps.tile([128, 512], F32, tag=f"acc{cc}h{h}", name=f"acc{cc}h{h}") for h in range(2)]
                  for cc in range(2)]

        for g in range(NGRP):
            for l, (H, W) in enumerate(FEATS_HW):
                HW = H * W
                KT = (HW + 127) // 128
                fsb = fpool.tile([128, KT * 256], BF16, tag="fsb")
                for kt in range(KT):
                    ksz = min(128, HW - kt * 128)
                    nc.sync.dma_start(fsb[:ksz, kt * 256:(kt + 1) * 256],
                                      t_F[(g, l)][kt * 128:kt * 128 + ksz, :])
                first_gl = (g == 0 and l == 0)
                last_gl = (g == NGRP - 1 and l == NL - 1)
                for half in range(2):
                    Ats = []
                    for mm in range(4):
                        m = half * 4 + mm
                        A = apool.tile([128, HW], BF16, tag="A")
                        Ats.append(A)
                        base = l * 24 + g * 8
                        dx = xpool.tile([128, GSL * MAXW], F32, tag="dx")
                        dxv = dx[:, :GSL * W].rearrange("p (s w) -> p s w", w=W)
                        nc.vector.tensor_tensor(
                            dxv,
                            iox[:, :W].unsqueeze(1).broadcast_to([128, GSL, W]),
                            pxn[m][:, base:base + GSL].unsqueeze(2).broadcast_to([128, GSL, W]),
                            ALU.add)
                        tx = xpool.tile([128, GSL * MAXW], BF16, tag="tx")
                        nc.scalar.activation(dx[:, :GSL * W], dx[:, :GSL * W], ACTF.Abs)
                        nc.scalar.activation(tx[:, :GSL * W], dx[:, :GSL * W], ACTF.Relu,
                                             bias=1.0, scale=-1.0)
                        dy = xpool.tile([128, GSL * MAXH], F32, tag="dy")
                        dyv = dy[:, :GSL * H].rearrange("p (s h) -> p s h", h=H)
                        nc.vector.tensor_tensor(
                            dyv,
                            ioy[:, :H].unsqueeze(1).broadcast_to([128, GSL, H]),
                            pyn[m][:, base:base + GSL].unsqueeze(2).broadcast_to([128, GSL, H]),
                            ALU.add)
                        ty = xpool.tile([128, GSL * MAXH], BF16, tag="ty")
                        nc.scalar.activation(dy[:, :GSL * H], dy[:, :GSL * H], ACTF.Abs)
                        nc.scalar.activation(ty[:, :GSL * H], dy[:, :GSL * H], ACTF.Relu,
                                             bias=1.0, scale=-1.0)
                        txv = tx[:, :GSL * W].rearrange("p (s w) -> p s w", w=W)
                        tyv = ty[:, :GSL * H].rearrange("p (s h) -> p s h", h=H)
                        if l == 0:
                            # strip form: all tensor operands packed stride-1
                            # (qualifies for the DVE 2x bf16 mode); the tmp
                            # outer-product op is eliminated entirely.
                            tya = xpool.tile([128, GSL * MAXH], F32, tag="tya")
                            for js in range(GSL):
                                aesc = aef[m][:, l * 24 + g * 8 + js:l * 24 + g * 8 + js + 1]
                                nc.vector.tensor_scalar(
                                    tya[:, js * H:(js + 1) * H],
                                    tyv[:, js], aesc, None, ALU.mult)
                            Av = A[:].rearrange("p (h w) -> p h w", w=W)
                            for y in range(H):
                                for js in range(GSL):
                                    ysc = tya[:, js * H + y:js * H + y + 1]
                                    if js == 0:
                                        nc.vector.tensor_scalar(
                                            Av[:, y], txv[:, js], ysc, None, ALU.mult)
                                    else:
                                        nc.vector.scalar_tensor_tensor(
                                            Av[:, y], txv[:, js], ysc, Av[:, y],
                                            ALU.mult, ALU.add)
                        else:
                            for js in range(GSL):
                                tmp = tpool.tile([128, HW], BF16, tag="tmp", bufs=1)
                                tmpv = tmp[:].rearrange("p (h w) -> p h w", w=W)
                                nc.vector.tensor_tensor(
                                    tmpv,
                                    txv[:, js].unsqueeze(1).broadcast_to([128, H, W]),
                                    tyv[:, js].unsqueeze(2).broadcast_to([128, H, W]),
                                    ALU.mult)
                                aesc = aef[m][:, l * 24 + g * 8 + js:l * 24 + g * 8 + js + 1]
                                if js == 0:
                                    nc.vector.tensor_scalar(A[:], tmp[:], aesc, None, ALU.mult)
                                else:
                                    nc.vector.scalar_tensor_tensor(
                                        A[:], tmp[:], aesc, A[:], ALU.mult, ALU.add)
                    for kt in range(KT):
                        ksz = min(128, HW - kt * 128)
                        AT = atp.tile([128, 512], BF16, tag="AT")
                        for mm in range(4):
                            tp = pspool.tile([128, 128], BF16, tag="tp", bufs=2)
                            nc.tensor.transpose(tp[:ksz, :],
                                                Ats[mm][:, kt * 128:kt * 128 + ksz],
                                                ident[:])
                            nc.scalar.copy(AT[:ksz, mm * 128:(mm + 1) * 128], tp[:ksz, :])
                        for cc in range(2):
                            nc.tensor.matmul(
                                acc_ps[cc][half][:],
                                fsb[:ksz, kt * 256 + cc * 128:kt * 256 + (cc + 1) * 128],
                                AT[:ksz, :],
                                start=(first_gl and kt == 0),
                                stop=(last_gl and kt == KT - 1))

        for cc in range(2):
            for half in range(2):
                nc.vector.tensor_copy(accT[cc][:, half * 512:(half + 1) * 512],
                                      acc_ps[cc][half][:])

        # ---- final linear + bias + residual ----
        qres = [ppool.tile([128, Q], F32, tag=f"qres{k}", name=f"qres{k}", bufs=1) for k in range(2)]
        for k in range(2):
            nc.sync.dma_start(qres[k][:], t_qresT[k * 128:(k + 1) * 128, :])
        for cc in range(2):
            for qc in range(2):
                op = pspool.tile([128, 512], F32, tag="scps", name="outp", bufs=2)
                for k in range(2):
                    nc.tensor.matmul(op[:],
                                     wout[k][:, cc * 128:(cc + 1) * 128],
                                     accT[k][:, qc * 512:(qc + 1) * 512],
                                     start=(k == 0), stop=(k == 1))
                ob = tpool.tile([128, 512], F32, tag="ob")
                nc.vector.scalar_tensor_tensor(
                    ob[:], op[:], boutc[cc][:, 0:1],
                    qres[cc][:, qc * 512:(qc + 1) * 512], ALU.add, ALU.add)
                nc.sync.dma_start(t_out[cc * 128:(cc + 1) * 128, qc * 512:(qc + 1) * 512],
                                  ob[:])
    nc.compile()
    return nc


def _prep_inputs(inp):
    query = np.ascontiguousarray(inp["query"][0], np.float32)
    qpos = np.ascontiguousarray(inp["query_pos"][0], np.float32)
    ref_q = np.transpose(inp["reference_points"], (0, 2, 3, 1, 4)).reshape(Q, NPIL, 3)
    ref_scaled = (ref_q * PC_SPAN + PC_LOW).astype(np.float32)
    L_all = np.asarray(inp["lidar2img"][0], np.float32)
    qT = np.ascontiguousarray(query.T)
    qposT = np.ascontiguousarray(qpos.T)
    iox = np.tile(np.arange(MAXW, dtype=np.float32), (128, 1))
    ioy = np.tile(np.arange(MAXH, dtype=np.float32), (128, 1))
    ident = np.eye(128, dtype=np.float32)
    ones = np.ones((1, Q), np.float32)
    Wout = np.ascontiguousarray(inp["W_out"], np.float32)
    boutC = np.ascontiguousarray(inp["b_out"].reshape(C, 1), np.float32)
    zeros_cq = np.zeros((C, Q), np.float32)

    in_maps = []
    for core in range(8):
        slots = [_slot_decode(core * NSLOT + j) for j in range(NSLOT)]
        W_off_c = np.zeros((C, 256), np.float32)
        b_off_c = np.zeros((1, 256), np.float32)
        W_attn_c = np.zeros((C, 128), np.float32)
        b_attn_c = np.zeros((1, 128), np.float32)
        refc = np.zeros((Q, NSLOT, 4, 3), np.float32)
        Lexp = np.zeros((12, NSLOT, 4), np.float32)
        for j, (n, h, pil, t) in enumerate(slots):
            for l in range(NL):
                for xy in range(2):
                    src = (((h * NL + l) * NPIL + pil) * NPT + t) * 2 + xy
                    dst = (j * NL + l) * 2 + xy
                    W_off_c[:, dst] = inp["W_off"][:, src]
                    b_off_c[0, dst] = inp["b_off"][src]
                srca = (h * PP + pil * NPT + t) * NL + l
                W_attn_c[:, j * NL + l] = inp["W_attn"][:, srca]
                b_attn_c[0, j * NL + l] = inp["b_attn"][srca]
            order = [pil] + [p for p in range(4) if p != pil]
            refc[:, j] = ref_scaled[:, order]
            for i in range(3):
                for k in range(4):
                    Lexp[i * 4 + k, j, :] = L_all[n][i, k]
        m = {
            "qT": qT, "qposT": qposT,
            "qresT": qT if core == 0 else zeros_cq,
            "Woff": W_off_c, "boff": b_off_c,
            "Wattn": W_attn_c, "battn": b_attn_c,
            "Wout": Wout,
            "boutC": boutC if core == 0 else np.zeros((C, 1), np.float32),
            "refS": np.ascontiguousarray(refc.reshape(Q, NSLOT * 12)),
            "Lexp": np.ascontiguousarray(
                np.tile(Lexp.reshape(1, 12 * NSLOT * 4), (128, 1))),
            "iox": iox, "ioy": ioy, "ones": ones,
            "ident": np.eye(128, dtype=np.float32).astype(__import__("ml_dtypes").bfloat16),
        }
        for g in range(NGRP):
            cam = (core * NGRP + g) // 4
            for l, (H, W) in enumerate(FEATS_HW):
                F = np.asarray(inp[f"feat{l}"][0, cam], np.float32).reshape(C, H * W)
                import ml_dtypes
                m[f"F{g}{l}"] = np.ascontiguousarray(F.T).astype(ml_dtypes.bfloat16)
        in_maps.append(m)
    return in_maps


_NC = None


def kernel(**inputs):
    global _NC
    inp = {k: np.asarray(v) for k, v in inputs.items()}
    if _NC is None:
        _NC = _build_program()
    in_maps = _prep_inputs(inp)
    res = run_bass_kernel_spmd(_NC, in_maps, core_ids=list(range(8)))
    out = np.zeros((C, Q), np.float32)
    for r in res.results:
        out += np.asarray(r["outT"], np.float32)
    return np.ascontiguousarray(out.T).reshape(1, Q, C)


# revision 13
# speedup vs baseline: 2238.2986x; 1.0049x over previous
"""BEVSDTransformerDecoder — Trainium2 Bass kernel (8-core SPMD).

Algorithm: multi-camera deformable attention, computed exactly (no gathers):
for each (camera, level) the sparse bilinear-sampling contraction is written
as  out^T += F^T(HW,C)^T-matmul with a dense weight matrix A(Q, HW) built on
DVE from triangle kernels: relu(1 - |iota - px|) is exactly the bilinear
weight profile of a sample at pixel coordinate px (zero padding automatic).

Sharding (uniform SPMD program): the 6 cams x 32 sample-slots = 192 global
slots are split into 24 single-camera groups of 8 slots; each of the 8 cores
processes 3 groups (24 slots) over all 4 levels.  Per-core weight-column
permutations (host-side layout prep of W_off/W_attn) select each core's
slots, so every core runs the identical program.  Host sums the per-core
partial outputs (the all-reduce of the masked scatter-add over cameras).
"""

import os
import sys
import numpy as np
from contextlib import ExitStack

sys.path.insert(0, "/opt/trn_rl_repo")

import concourse.bass as bass
import concourse.bacc as bacc
import concourse.tile as tile
from concourse import mybir
from concourse.bass_utils import run_bass_kernel_spmd

F32 = mybir.dt.float32
BF16 = mybir.dt.bfloat16
ALU = mybir.AluOpType
ACTF = mybir.ActivationFunctionType

NH, NL, NPIL, NPT = 4, 4, 4, 2
PP = NPIL * NPT
Q, C, NCAM = 1024, 256, 6
IMG_H, IMG_W, EPS = 256.0, 704.0, 1e-5
PC_LOW = np.array([-51.2, -51.2, -5.0], np.float32)
PC_SPAN = np.array([102.4, 102.4, 8.0], np.float32)
FEATS_HW = [(32, 88), (16, 44), (8, 22), (4, 11)]
NSLOT = 24          # slots per core
NGRP = 3            # single-camera groups of 8 slots per core
GSL = 8             # slots per group
NQT = 8             # q tiles of 128
MAXW, MAXH = 88, 32


def _slot_decode(gid):
    n, s = gid // 32, gid % 32
    return n, s // 8, (s % 8) // 2, s % 2      # cam, head, pillar, point


_MAKESPAN_NS = None


def _build_program():
    global _MAKESPAN_NS
    import concourse.bass_interp as _bi
    _orig_sim = _bi.CoreSim.simulate
    _times = []

    def _patched(self, *a, **k):
        r = _orig_sim(self, *a, **k)
        try:
            _times.append(int(self.time))
        except Exception:
            pass
        return r

    _bi.CoreSim.simulate = _patched
    try:
        nc = _build_program_inner()
    finally:
        _bi.CoreSim.simulate = _orig_sim
    if _times:
        _MAKESPAN_NS = max(_times)
    return nc


def _build_program_inner():
    nc = bacc.Bacc("TRN2", target_bir_lowering=False, debug=False, num_devices=8)
    dp = nc.declare_dram_parameter
    t_qT = dp("qT", [C, Q], F32, isOutput=False)
    t_qpT = dp("qposT", [C, Q], F32, isOutput=False)
    t_qresT = dp("qresT", [C, Q], F32, isOutput=False)
    t_Woff = dp("Woff", [C, 256], F32, isOutput=False)
    t_boff = dp("boff", [1, 256], F32, isOutput=False)
    t_Wattn = dp("Wattn", [C, 128], F32, isOutput=False)
    t_battn = dp("battn", [1, 128], F32, isOutput=False)
    t_Wout = dp("Wout", [C, C], F32, isOutput=False)
    t_bout = dp("boutC", [C, 1], F32, isOutput=False)
    t_ref = dp("refS", [Q, NSLOT * 12], F32, isOutput=False)
    t_Lexp = dp("Lexp", [128, 12 * NSLOT * 4], F32, isOutput=False)
    t_iox = dp("iox", [128, MAXW], F32, isOutput=False)
    t_ioy = dp("ioy", [128, MAXH], F32, isOutput=False)
    t_id = dp("ident", [128, 128], BF16, isOutput=False)
    t_ones = dp("ones", [1, Q], F32, isOutput=False)
    t_F = {}
    for g in range(NGRP):
        for l, (H, W) in enumerate(FEATS_HW):
            t_F[(g, l)] = dp(f"F{g}{l}", [H * W, C], BF16, isOutput=False)
    t_out = dp("outT", [C, Q], F32, isOutput=True)

    with tile.TileContext(nc) as tc, ExitStack() as ctx:
        cpool = ctx.enter_context(tc.tile_pool(name="consts", bufs=1))
        ppool = ctx.enter_context(tc.tile_pool(name="proj", bufs=2))
        apool = ctx.enter_context(tc.tile_pool(name="A", bufs=6))
        tpool = ctx.enter_context(tc.tile_pool(name="tmp", bufs=2))
        xpool = ctx.enter_context(tc.tile_pool(name="tri", bufs=4))
        fpool = ctx.enter_context(tc.tile_pool(name="feat", bufs=2))
        atp = ctx.enter_context(tc.tile_pool(name="AT", bufs=3))
        pspool = ctx.enter_context(tc.tile_pool(name="ps", bufs=2, space="PSUM"))
        accps = ctx.enter_context(tc.tile_pool(name="accps", bufs=1, space="PSUM"))

        # ---- load constants ----
        def load(shape, src, name):
            t = cpool.tile(shape, F32, tag=name, name=name)
            nc.sync.dma_start(t[:], src)
            return t

        woff = [load([128, 256], t_Woff[k * 128:(k + 1) * 128, :], f"woff{k}") for k in range(2)]
        wattn = [load([128, 128], t_Wattn[k * 128:(k + 1) * 128, :], f"wattn{k}") for k in range(2)]
        wout = [load([128, 256], t_Wout[k * 128:(k + 1) * 128, :], f"wout{k}") for k in range(2)]
        boutc = [load([128, 1], t_bout[k * 128:(k + 1) * 128, :], f"bout{k}") for k in range(2)]
        boff = load([1, 256], t_boff[:, :], "boff")
        battn = load([1, 128], t_battn[:, :], "battn")
        lexp = load([128, 12 * 96], t_Lexp[:, :], "lexp")
        iox = load([128, MAXW], t_iox[:, :], "iox")
        ioy = load([128, MAXH], t_ioy[:, :], "ioy")
        ident = cpool.tile([128, 128], BF16, tag="ident", name="ident")
        nc.sync.dma_start(ident[:], t_id[:, :])
        ones = load([1, Q], t_ones[:, :], "ones")

        # qp^T = (query + query_pos)^T   [2 x (128, 1024)]
        qpT = []
        for k in range(2):
            a = ppool.tile([128, Q], F32, tag="qld", bufs=1)
            b = ppool.tile([128, Q], F32, tag="qld2", bufs=1)
            nc.sync.dma_start(a[:], t_qT[k * 128:(k + 1) * 128, :])
            nc.sync.dma_start(b[:], t_qpT[k * 128:(k + 1) * 128, :])
            s = cpool.tile([128, Q], F32, tag=f"qpT{k}")
            nc.vector.tensor_add(s[:], a[:], b[:])
            qpT.append(s)

        # per-q-tile persistent small tensors
        pxn = [cpool.tile([128, 96], F32, tag=f"pxn{m}", name=f"pxn{m}") for m in range(NQT)]
        pyn = [cpool.tile([128, 96], F32, tag=f"pyn{m}", name=f"pyn{m}") for m in range(NQT)]
        aef = [cpool.tile([128, 96], F32, tag=f"aef{m}", name=f"aef{m}") for m in range(NQT)]

        # ---- per q-tile: linear layers + projection ----
        for m in range(NQT):
            qsl = slice(m * 128, (m + 1) * 128)
            offp = pspool.tile([128, 256], F32, tag="scps", name="offp", bufs=2, padded_shape=[128, 512])
            for k in range(2):
                nc.tensor.matmul(offp[:], qpT[k][:, qsl], woff[k][:],
                                 start=(k == 0), stop=False)
            nc.tensor.matmul(offp[:], ones[:, qsl], boff[:],
                             start=False, stop=True)
            attp = pspool.tile([128, 128], F32, tag="scps", name="attp", bufs=2, padded_shape=[128, 512])
            for k in range(2):
                nc.tensor.matmul(attp[:], qpT[k][:, qsl], wattn[k][:],
                                 start=(k == 0), stop=False)
            nc.tensor.matmul(attp[:], ones[:, qsl], battn[:],
                             start=False, stop=True)
            off_sb = ppool.tile([128, 256], F32, tag="offsb")
            nc.scalar.copy(off_sb[:], offp[:])
            attnw = ppool.tile([128, 128], F32, tag="attnw")
            nc.scalar.activation(attnw[:], attp[:], ACTF.Sigmoid)

            refm = ppool.tile([128, NSLOT * 12], F32, tag="refm")
            nc.sync.dma_start(refm[:], t_ref[qsl, :])
            r3 = refm[:].rearrange("p (s c) -> p s c", c=3)
            X, Y, Z = r3[:, :, 0], r3[:, :, 1], r3[:, :, 2]

            def LP(i):
                return lexp[:, i * 96:(i + 1) * 96]

            uvd = []
            for comp in range(3):
                acc = ppool.tile([128, 96], F32, tag=f"uvd{comp}", name=f"uvd{comp}", bufs=1)
                nc.vector.tensor_mul(acc[:], X, LP(comp * 4 + 0))
                tmp2 = ppool.tile([128, 96], F32, tag="projtmp")
                nc.vector.tensor_mul(tmp2[:], Y, LP(comp * 4 + 1))
                nc.vector.tensor_add(acc[:], acc[:], tmp2[:])
                nc.vector.tensor_mul(tmp2[:], Z, LP(comp * 4 + 2))
                nc.vector.tensor_add(acc[:], acc[:], tmp2[:])
                nc.vector.tensor_add(acc[:], acc[:], LP(comp * 4 + 3))
                uvd.append(acc)
            u, v, d = uvd
            dcl = ppool.tile([128, 96], F32, tag="dcl")
            nc.vector.tensor_scalar(dcl[:], d[:], float(EPS), None, ALU.max)
            val = ppool.tile([128, 96], F32, tag="val")
            nc.vector.tensor_scalar(val[:], d[:], float(EPS), None, ALU.is_gt)
            tmpv = ppool.tile([128, 96], F32, tag="tmpv")
            nc.vector.tensor_scalar(tmpv[:], u[:], 0.0, None, ALU.is_gt)
            nc.vector.tensor_mul(val[:], val[:], tmpv[:])
            nc.vector.tensor_scalar(tmpv[:], v[:], 0.0, None, ALU.is_gt)
            nc.vector.tensor_mul(val[:], val[:], tmpv[:])
            lim = ppool.tile([128, 96], F32, tag="lim")
            nc.vector.tensor_scalar(lim[:], dcl[:], float(IMG_W), None, ALU.mult)
            nc.vector.tensor_tensor(tmpv[:], u[:], lim[:], ALU.is_lt)
            nc.vector.tensor_mul(val[:], val[:], tmpv[:])
            nc.vector.tensor_scalar(lim[:], dcl[:], float(IMG_H), None, ALU.mult)
            nc.vector.tensor_tensor(tmpv[:], v[:], lim[:], ALU.is_lt)
            nc.vector.tensor_mul(val[:], val[:], tmpv[:])
            qmask = ppool.tile([128, 24], F32, tag="qmask")
            nc.vector.tensor_reduce(qmask[:], val[:].rearrange("p (s r) -> p s r", r=4),
                                    mybir.AxisListType.X, ALU.max)

            # own-pillar grid coords
            rec = ppool.tile([128, 24], F32, tag="rec")
            d4 = dcl[:].rearrange("p (s r) -> p s r", r=4)
            nc.vector.reciprocal(rec[:], d4[:, :, 0])
            gx = ppool.tile([128, 24], F32, tag="gx")
            u4 = u[:].rearrange("p (s r) -> p s r", r=4)
            nc.vector.tensor_mul(gx[:], u4[:, :, 0], rec[:])
            nc.vector.tensor_scalar(gx[:], gx[:], float(2.0 / IMG_W), -1.0, ALU.mult, ALU.add)
            gy = ppool.tile([128, 24], F32, tag="gy")
            v4 = v[:].rearrange("p (s r) -> p s r", r=4)
            nc.vector.tensor_mul(gy[:], v4[:, :, 0], rec[:])
            nc.vector.tensor_scalar(gy[:], gy[:], float(2.0 / IMG_H), -1.0, ALU.mult, ALU.add)

            offr = off_sb[:].rearrange("p (j r) -> p j r", r=8)
            attr = attnw[:].rearrange("p (j r) -> p j r", r=4)
            for l, (H, W) in enumerate(FEATS_HW):
                lsl = slice(l * 24, (l + 1) * 24)
                sx = ppool.tile([128, 24], F32, tag="sx")
                nc.vector.tensor_add(sx[:], gx[:], offr[:, :24, 2 * l + 0])
                nc.vector.tensor_scalar(pxn[m][:, lsl], sx[:], float(-W / 2.0),
                                        float(0.5 - W / 2.0), ALU.mult, ALU.add)
                nc.vector.tensor_add(sx[:], gy[:], offr[:, :24, 2 * l + 1])
                nc.vector.tensor_scalar(pyn[m][:, lsl], sx[:], float(-H / 2.0),
                                        float(0.5 - H / 2.0), ALU.mult, ALU.add)
                nc.vector.tensor_tensor(aef[m][:, lsl], attr[:, :24, l], qmask[:], ALU.mult)

        # ---- main build + matmul ----
        accT = [cpool.tile([128, Q], F32, tag=f"accT{k}", name=f"accT{k}") for k in range(2)]
        acc_ps = [[accps.tile([128, 512], F32, tag=f"acc{cc}h{h}", name=f"acc{cc}h{h}") for h in range(2)]
                  for cc in range(2)]

        for g in range(NGRP):
            for l, (H, W) in enumerate(FEATS_HW):
                HW = H * W
                KT = (HW + 127) // 128
                fsb = fpool.tile([128, KT * 256], BF16, tag="fsb")
                for kt in range(KT):
                    ksz = min(128, HW - kt * 128)
                    nc.sync.dma_start(fsb[:ksz, kt * 256:(kt + 1) * 256],
                                      t_F[(g, l)][kt * 128:kt * 128 + ksz, :])
                first_gl = (g == 0 and l == 0)
                last_gl = (g == NGRP - 1 and l == NL - 1)
                for half in range(2):
                    Ats = []
                    for mm in range(4):
                        m = half * 4 + mm
                        A = apool.tile([128, HW], BF16, tag="A")
                        Ats.append(A)
                        base = l * 24 + g * 8
                        dx = xpool.tile([128, GSL * MAXW], F32, tag="dx")
                        dxv = dx[:, :GSL * W].rearrange("p (s w) -> p s w", w=W)
                        nc.vector.tensor_tensor(
                            dxv,
                            iox[:, :W].unsqueeze(1).broadcast_to([128, GSL, W]),
                            pxn[m][:, base:base + GSL].unsqueeze(2).broadcast_to([128, GSL, W]),
                            ALU.add)
                        tx = xpool.tile([128, GSL * MAXW], BF16, tag="tx")
                        nc.scalar.activation(dx[:, :GSL * W], dx[:, :GSL * W], ACTF.Abs)
                        nc.scalar.activation(tx[:, :GSL * W], dx[:, :GSL * W], ACTF.Relu,
                                             bias=1.0, scale=-1.0)
                        dy = xpool.tile([128, GSL * MAXH], F32, tag="dy")
                        dyv = dy[:, :GSL * H].rearrange("p (s h) -> p s h", h=H)
                        nc.vector.tensor_tensor(
                            dyv,
                            ioy[:, :H].unsqueeze(1).broadcast_to([128, GSL, H]),
                            pyn[m][:, base:base + GSL].unsqueeze(2).broadcast_to([128, GSL, H]),
                            ALU.add)
                        ty = xpool.tile([128, GSL * MAXH], BF16, tag="ty")
                        nc.scalar.activation(dy[:, :GSL * H], dy[:, :GSL * H], ACTF.Abs)
                        nc.scalar.activation(ty[:, :GSL * H], dy[:, :GSL * H], ACTF.Relu,
                                             bias=1.0, scale=-1.0)
                        txv = tx[:, :GSL * W].rearrange("p (s w) -> p s w", w=W)
                        tyv = ty[:, :GSL * H].rearrange("p (s h) -> p s h", h=H)
                        if l == 0:
                            # strip form: all tensor operands packed stride-1
                            # (qualifies for the DVE 2x bf16 mode); the tmp
                            # outer-product op is eliminated entirely.
                            tya = xpool.tile([128, GSL * MAXH], F32, tag="tya")
                            for js in range(GSL):
                                aesc = aef[m][:, l * 24 + g * 8 + js:l * 24 + g * 8 + js + 1]
                                nc.vector.tensor_scalar(
                                    tya[:, js * H:(js + 1) * H],
                                    tyv[:, js], aesc, None, ALU.mult)
                            Av = A[:].rearrange("p (h w) -> p h w", w=W)
                            for y in range(H):
                                for js in range(GSL):
                                    ysc = tya[:, js * H + y:js * H + y + 1]
                                    if js == 0:
                                        nc.vector.tensor_scalar(
                                            Av[:, y], txv[:, js], ysc, None, ALU.mult)
                                    else:
                                        nc.vector.scalar_tensor_tensor(
                                            Av[:, y], txv[:, js], ysc, Av[:, y],
                                            ALU.mult, ALU.add)
                        else:
                            for js in range(GSL):
                                tmp = tpool.tile([128, HW], BF16, tag="tmp", bufs=1)
                                tmpv = tmp[:].rearrange("p (h w) -> p h w", w=W)
                                nc.vector.tensor_tensor(
                                    tmpv,
                                    txv[:, js].unsqueeze(1).broadcast_to([128, H, W]),
                                    tyv[:, js].unsqueeze(2).broadcast_to([128, H, W]),
                                    ALU.mult)
                                aesc = aef[m][:, l * 24 + g * 8 + js:l * 24 + g * 8 + js + 1]
                                if js == 0:
                                    nc.vector.tensor_scalar(A[:], tmp[:], aesc, None, ALU.mult)
                                else:
                                    nc.vector.scalar_tensor_tensor(
                                        A[:], tmp[:], aesc, A[:], ALU.mult, ALU.add)
                    for kt in range(KT):
                        ksz = min(128, HW - kt * 128)
                        AT = atp.tile([128, 512], BF16, tag="AT")
                        for mm in range(4):
                            tp = pspool.tile([128, 128], BF16, tag="tp", bufs=2)
                            nc.tensor.transpose(tp[:ksz, :],
                                                Ats[mm][:, kt * 128:kt * 128 + ksz],
                                                ident[:])
                            nc.scalar.copy(AT[:ksz, mm * 128:(mm + 1) * 128], tp[:ksz, :])
                        for cc in range(2):
                            nc.tensor.matmul(
                                acc_ps[cc][half][:],
                                fsb[:ksz, kt * 256 + cc * 128:kt * 256 + (cc + 1) * 128],
                                AT[:ksz, :],
                                start=(first_gl and kt == 0),
                                stop=(last_gl and kt == KT - 1))

        for cc in range(2):
            for half in range(2):
                nc.vector.tensor_copy(accT[cc][:, half * 512:(half + 1) * 512],
                                      acc_ps[cc][half][:])

        # ---- final linear + bias + residual ----
        qres = [ppool.tile([128, Q], F32, tag=f"qres{k}", name=f"qres{k}", bufs=1) for k in range(2)]
        for k in range(2):
            nc.sync.dma_start(qres[k][:], t_qresT[k * 128:(k + 1) * 128, :])
        for cc in range(2):
            for qc in range(2):
                op = pspool.tile([128, 512], F32, tag="scps", name="outp", bufs=2)
                for k in range(2):
                    nc.tensor.matmul(op[:],
                                     wout[k][:, cc * 128:(cc + 1) * 128],
                                     accT[k][:, qc * 512:(qc + 1) * 512],
                                     start=(k == 0), stop=(k == 1))
                ob = tpool.tile([128, 512], F32, tag="ob")
                nc.vector.scalar_tensor_tensor(
                    ob[:], op[:], boutc[cc][:, 0:1],
                    qres[cc][:, qc * 512:(qc + 1) * 512], ALU.add, ALU.add)
                nc.sync.dma_start(t_out[cc * 128:(cc + 1) * 128, qc * 512:(qc + 1) * 512],
                                  ob[:])
    nc.compile()
    return nc


def _prep_inputs(inp):
    query = np.ascontiguousarray(inp["query"][0], np.float32)
    qpos = np.ascontiguousarray(inp["query_pos"][0], np.float32)
    ref_q = np.transpose(inp["reference_points"], (0, 2, 3, 1, 4)).reshape(Q, NPIL, 3)
    ref_scaled = (ref_q * PC_SPAN + PC_LOW).astype(np.float32)
    L_all = np.asarray(inp["lidar2img"][0], np.float32)
    qT = np.ascontiguousarray(query.T)
    qposT = np.ascontiguousarray(qpos.T)
    iox = np.tile(np.arange(MAXW, dtype=np.float32), (128, 1))
    ioy = np.tile(np.arange(MAXH, dtype=np.float32), (128, 1))
    ident = np.eye(128, dtype=np.float32)
    ones = np.ones((1, Q), np.float32)
    Wout = np.ascontiguousarray(inp["W_out"], np.float32)
    boutC = np.ascontiguousarray(inp["b_out"].reshape(C, 1), np.float32)
    zeros_cq = np.zeros((C, Q), np.float32)

    in_maps = []
    for core in range(8):
        slots = [_slot_decode(core * NSLOT + j) for j in range(NSLOT)]
        W_off_c = np.zeros((C, 256), np.float32)
        b_off_c = np.zeros((1, 256), np.float32)
        W_attn_c = np.zeros((C, 128), np.float32)
        b_attn_c = np.zeros((1, 128), np.float32)
        refc = np.zeros((Q, NSLOT, 4, 3), np.float32)
        Lexp = np.zeros((12, NSLOT, 4), np.float32)
        for j, (n, h, pil, t) in enumerate(slots):
            for l in range(NL):
                for xy in range(2):
                    src = (((h * NL + l) * NPIL + pil) * NPT + t) * 2 + xy
                    dst = (j * NL + l) * 2 + xy
                    W_off_c[:, dst] = inp["W_off"][:, src]
                    b_off_c[0, dst] = inp["b_off"][src]
                srca = (h * PP + pil * NPT + t) * NL + l
                W_attn_c[:, j * NL + l] = inp["W_attn"][:, srca]
                b_attn_c[0, j * NL + l] = inp["b_attn"][srca]
            order = [pil] + [p for p in range(4) if p != pil]
            refc[:, j] = ref_scaled[:, order]
            for i in range(3):
                for k in range(4):
                    Lexp[i * 4 + k, j, :] = L_all[n][i, k]
        m = {
            "qT": qT, "qposT": qposT,
            "qresT": qT if core == 0 else zeros_cq,
            "Woff": W_off_c, "boff": b_off_c,
            "Wattn": W_attn_c, "battn": b_attn_c,
            "Wout": Wout,
            "boutC": boutC if core == 0 else np.zeros((C, 1), np.float32),
            "refS": np.ascontiguousarray(refc.reshape(Q, NSLOT * 12)),
            "Lexp": np.ascontiguousarray(
                np.tile(Lexp.reshape(1, 12 * NSLOT * 4), (128, 1))),
            "iox": iox, "ioy": ioy, "ones": ones,
            "ident": np.eye(128, dtype=np.float32).astype(__import__("ml_dtypes").bfloat16),
        }
        for g in range(NGRP):
            cam = (core * NGRP + g) // 4
            for l, (H, W) in enumerate(FEATS_HW):
                F = np.asarray(inp[f"feat{l}"][0, cam], np.float32).reshape(C, H * W)
                import ml_dtypes
                m[f"F{g}{l}"] = np.ascontiguousarray(F.T).astype(ml_dtypes.bfloat16)
        in_maps.append(m)
    return in_maps


_NC = None


def kernel(**inputs):
    global _NC
    inp = {k: np.asarray(v) for k, v in inputs.items()}
    if _NC is None:
        _NC = _build_program()
    in_maps = _prep_inputs(inp)
    res = run_bass_kernel_spmd(_NC, in_maps, core_ids=list(range(8)))
    out = np.zeros((C, Q), np.float32)
    for r in res.results:
        out += np.asarray(r["outT"], np.float32)
    return np.ascontiguousarray(out.T).reshape(1, Q, C)


# revision 17
# speedup vs baseline: 2297.0888x; 1.0263x over previous
"""BEVSDTransformerDecoder — Trainium2 Bass kernel (8-core SPMD).

Algorithm: multi-camera deformable attention, computed exactly (no gathers):
for each (camera, level) the sparse bilinear-sampling contraction is written
as  out^T += F^T(HW,C)^T-matmul with a dense weight matrix A(Q, HW) built on
DVE from triangle kernels: relu(1 - |iota - px|) is exactly the bilinear
weight profile of a sample at pixel coordinate px (zero padding automatic).

Sharding (uniform SPMD program): the 6 cams x 32 sample-slots = 192 global
slots are split into 24 single-camera groups of 8 slots; each of the 8 cores
processes 3 groups (24 slots) over all 4 levels.  Per-core weight-column
permutations (host-side layout prep of W_off/W_attn) select each core's
slots, so every core runs the identical program.  Host sums the per-core
partial outputs (the all-reduce of the masked scatter-add over cameras).
"""

import os
import sys
import numpy as np
from contextlib import ExitStack

sys.path.insert(0, "/opt/trn_rl_repo")

import concourse.bass as bass
import concourse.bacc as bacc
import concourse.tile as tile
from concourse import mybir
from concourse.bass_utils import run_bass_kernel_spmd

F32 = mybir.dt.float32
BF16 = mybir.dt.bfloat16
ALU = mybir.AluOpType
ACTF = mybir.ActivationFunctionType

NH, NL, NPIL, NPT = 4, 4, 4, 2
PP = NPIL * NPT
Q, C, NCAM = 1024, 256, 6
IMG_H, IMG_W, EPS = 256.0, 704.0, 1e-5
PC_LOW = np.array([-51.2, -51.2, -5.0], np.float32)
PC_SPAN = np.array([102.4, 102.4, 8.0], np.float32)
FEATS_HW = [(32, 88), (16, 44), (8, 22), (4, 11)]
NSLOT = 24          # slots per core
NGRP = 3            # single-camera groups of 8 slots per core
GSL = 8             # slots per group
NQT = 8             # q tiles of 128
MAXW, MAXH = 88, 32


def _slot_decode(gid):
    n, s = gid // 32, gid % 32
    return n, s // 8, (s % 8) // 2, s % 2      # cam, head, pillar, point


_MAKESPAN_NS = None


def _build_program():
    global _MAKESPAN_NS
    import concourse.bass_interp as _bi
    _orig_sim = _bi.CoreSim.simulate
    _times = []

    def _patched(self, *a, **k):
        r = _orig_sim(self, *a, **k)
        try:
            _times.append(int(self.time))
        except Exception:
            pass
        return r

    _bi.CoreSim.simulate = _patched
    try:
        nc = _build_program_inner()
    finally:
        _bi.CoreSim.simulate = _orig_sim
    if _times:
        _MAKESPAN_NS = max(_times)
    return nc


def _build_program_inner():
    nc = bacc.Bacc("TRN2", target_bir_lowering=False, debug=False, num_devices=8)
    dp = nc.declare_dram_parameter
    t_qT = dp("qT", [C, Q], F32, isOutput=False)
    t_qpT = dp("qposT", [C, Q], F32, isOutput=False)
    t_qresT = dp("qresT", [C, Q], F32, isOutput=False)
    t_Woff = dp("Woff", [C, 256], F32, isOutput=False)
    t_boff = dp("boff", [1, 256], F32, isOutput=False)
    t_Wattn = dp("Wattn", [C, 128], F32, isOutput=False)
    t_battn = dp("battn", [1, 128], F32, isOutput=False)
    t_Wout = dp("Wout", [C, C], F32, isOutput=False)
    t_bout = dp("boutC", [C, 1], F32, isOutput=False)
    t_ref = dp("refS", [Q, NSLOT * 12], F32, isOutput=False)
    t_Lexp = dp("Lexp", [128, 12 * NSLOT * 4], F32, isOutput=False)
    t_iox = dp("iox", [128, MAXW], F32, isOutput=False)
    t_ioy = dp("ioy", [128, MAXH], F32, isOutput=False)
    t_id = dp("ident", [128, 128], BF16, isOutput=False)
    t_ones = dp("ones", [1, Q], F32, isOutput=False)
    t_F = {}
    for g in range(NGRP):
        for l, (H, W) in enumerate(FEATS_HW):
            t_F[(g, l)] = dp(f"F{g}{l}", [H * W, C], BF16, isOutput=False)
    t_out = dp("outT", [C, Q], F32, isOutput=True)

    with tile.TileContext(nc) as tc, ExitStack() as ctx:
        cpool = ctx.enter_context(tc.tile_pool(name="consts", bufs=1))
        ppool = ctx.enter_context(tc.tile_pool(name="proj", bufs=2))
        apool = ctx.enter_context(tc.tile_pool(name="A", bufs=6))
        tpool = ctx.enter_context(tc.tile_pool(name="tmp", bufs=2))
        xpool = ctx.enter_context(tc.tile_pool(name="tri", bufs=4))
        fpool = ctx.enter_context(tc.tile_pool(name="feat", bufs=2))
        atp = ctx.enter_context(tc.tile_pool(name="AT", bufs=3))
        pspool = ctx.enter_context(tc.tile_pool(name="ps", bufs=2, space="PSUM"))
        accps = ctx.enter_context(tc.tile_pool(name="accps", bufs=1, space="PSUM"))

        # ---- load constants ----
        def load(shape, src, name):
            t = cpool.tile(shape, F32, tag=name, name=name)
            nc.sync.dma_start(t[:], src)
            return t

        woff = [load([128, 256], t_Woff[k * 128:(k + 1) * 128, :], f"woff{k}") for k in range(2)]
        wattn = [load([128, 128], t_Wattn[k * 128:(k + 1) * 128, :], f"wattn{k}") for k in range(2)]
        wout = [load([128, 256], t_Wout[k * 128:(k + 1) * 128, :], f"wout{k}") for k in range(2)]
        boutc = [load([128, 1], t_bout[k * 128:(k + 1) * 128, :], f"bout{k}") for k in range(2)]
        boff = load([1, 256], t_boff[:, :], "boff")
        battn = load([1, 128], t_battn[:, :], "battn")
        lexp = load([128, 12 * 96], t_Lexp[:, :], "lexp")
        iox = load([128, MAXW], t_iox[:, :], "iox")
        ioy = load([128, MAXH], t_ioy[:, :], "ioy")
        ident = cpool.tile([128, 128], BF16, tag="ident", name="ident")
        nc.sync.dma_start(ident[:], t_id[:, :])
        ones = load([1, Q], t_ones[:, :], "ones")

        # qp^T = (query + query_pos)^T   [2 x (128, 1024)]
        qpT = []
        for k in range(2):
            a = ppool.tile([128, Q], F32, tag="qld", bufs=1)
            b = ppool.tile([128, Q], F32, tag="qld2", bufs=1)
            nc.sync.dma_start(a[:], t_qT[k * 128:(k + 1) * 128, :])
            nc.sync.dma_start(b[:], t_qpT[k * 128:(k + 1) * 128, :])
            s = cpool.tile([128, Q], F32, tag=f"qpT{k}")
            nc.vector.tensor_add(s[:], a[:], b[:])
            qpT.append(s)

        # per-q-tile persistent small tensors
        pxn = [cpool.tile([128, 96], F32, tag=f"pxn{m}", name=f"pxn{m}") for m in range(NQT)]
        pyn = [cpool.tile([128, 96], F32, tag=f"pyn{m}", name=f"pyn{m}") for m in range(NQT)]
        aef = [cpool.tile([128, 96], F32, tag=f"aef{m}", name=f"aef{m}") for m in range(NQT)]

        # ---- per q-tile: linear layers + projection ----
        for m in range(NQT):
            qsl = slice(m * 128, (m + 1) * 128)
            offp = pspool.tile([128, 256], F32, tag="scps", name="offp", bufs=2, padded_shape=[128, 512])
            for k in range(2):
                nc.tensor.matmul(offp[:], qpT[k][:, qsl], woff[k][:],
                                 start=(k == 0), stop=False)
            nc.tensor.matmul(offp[:], ones[:, qsl], boff[:],
                             start=False, stop=True)
            attp = pspool.tile([128, 128], F32, tag="scps", name="attp", bufs=2, padded_shape=[128, 512])
            for k in range(2):
                nc.tensor.matmul(attp[:], qpT[k][:, qsl], wattn[k][:],
                                 start=(k == 0), stop=False)
            nc.tensor.matmul(attp[:], ones[:, qsl], battn[:],
                             start=False, stop=True)
            off_sb = ppool.tile([128, 256], F32, tag="offsb")
            nc.scalar.copy(off_sb[:], offp[:])
            attnw = ppool.tile([128, 128], F32, tag="attnw")
            nc.scalar.activation(attnw[:], attp[:], ACTF.Sigmoid)

            refm = ppool.tile([128, NSLOT * 12], F32, tag="refm")
            nc.sync.dma_start(refm[:], t_ref[qsl, :])
            r3 = refm[:].rearrange("p (s c) -> p s c", c=3)
            X, Y, Z = r3[:, :, 0], r3[:, :, 1], r3[:, :, 2]

            def LP(i):
                return lexp[:, i * 96:(i + 1) * 96]

            uvd = []
            for comp in range(3):
                acc = ppool.tile([128, 96], F32, tag=f"uvd{comp}", name=f"uvd{comp}", bufs=1)
                nc.vector.tensor_mul(acc[:], X, LP(comp * 4 + 0))
                tmp2 = ppool.tile([128, 96], F32, tag="projtmp")
                nc.vector.tensor_mul(tmp2[:], Y, LP(comp * 4 + 1))
                nc.vector.tensor_add(acc[:], acc[:], tmp2[:])
                nc.vector.tensor_mul(tmp2[:], Z, LP(comp * 4 + 2))
                nc.vector.tensor_add(acc[:], acc[:], tmp2[:])
                nc.vector.tensor_add(acc[:], acc[:], LP(comp * 4 + 3))
                uvd.append(acc)
            u, v, d = uvd
            dcl = ppool.tile([128, 96], F32, tag="dcl")
            nc.vector.tensor_scalar(dcl[:], d[:], float(EPS), None, ALU.max)
            val = ppool.tile([128, 96], F32, tag="val")
            nc.vector.tensor_scalar(val[:], d[:], float(EPS), None, ALU.is_gt)
            tmpv = ppool.tile([128, 96], F32, tag="tmpv")
            nc.vector.tensor_scalar(tmpv[:], u[:], 0.0, None, ALU.is_gt)
            nc.vector.tensor_mul(val[:], val[:], tmpv[:])
            nc.vector.tensor_scalar(tmpv[:], v[:], 0.0, None, ALU.is_gt)
            nc.vector.tensor_mul(val[:], val[:], tmpv[:])
            lim = ppool.tile([128, 96], F32, tag="lim")
            nc.vector.tensor_scalar(lim[:], dcl[:], float(IMG_W), None, ALU.mult)
            nc.vector.tensor_tensor(tmpv[:], u[:], lim[:], ALU.is_lt)
            nc.vector.tensor_mul(val[:], val[:], tmpv[:])
            nc.vector.tensor_scalar(lim[:], dcl[:], float(IMG_H), None, ALU.mult)
            nc.vector.tensor_tensor(tmpv[:], v[:], lim[:], ALU.is_lt)
            nc.vector.tensor_mul(val[:], val[:], tmpv[:])
            qmask = ppool.tile([128, 24], F32, tag="qmask")
            nc.vector.tensor_reduce(qmask[:], val[:].rearrange("p (s r) -> p s r", r=4),
                                    mybir.AxisListType.X, ALU.max)

            # own-pillar grid coords
            rec = ppool.tile([128, 24], F32, tag="rec")
            d4 = dcl[:].rearrange("p (s r) -> p s r", r=4)
            nc.vector.reciprocal(rec[:], d4[:, :, 0])
            gx = ppool.tile([128, 24], F32, tag="gx")
            u4 = u[:].rearrange("p (s r) -> p s r", r=4)
            nc.vector.tensor_mul(gx[:], u4[:, :, 0], rec[:])
            nc.vector.tensor_scalar(gx[:], gx[:], float(2.0 / IMG_W), -1.0, ALU.mult, ALU.add)
            gy = ppool.tile([128, 24], F32, tag="gy")
            v4 = v[:].rearrange("p (s r) -> p s r", r=4)
            nc.vector.tensor_mul(gy[:], v4[:, :, 0], rec[:])
            nc.vector.tensor_scalar(gy[:], gy[:], float(2.0 / IMG_H), -1.0, ALU.mult, ALU.add)

            offr = off_sb[:].rearrange("p (j r) -> p j r", r=8)
            attr = attnw[:].rearrange("p (j r) -> p j r", r=4)
            for l, (H, W) in enumerate(FEATS_HW):
                lsl = slice(l * 24, (l + 1) * 24)
                sx = ppool.tile([128, 24], F32, tag="sx")
                nc.vector.tensor_add(sx[:], gx[:], offr[:, :24, 2 * l + 0])
                nc.vector.tensor_scalar(pxn[m][:, lsl], sx[:], float(-W / 2.0),
                                        float(0.5 - W / 2.0), ALU.mult, ALU.add)
                nc.vector.tensor_add(sx[:], gy[:], offr[:, :24, 2 * l + 1])
                nc.vector.tensor_scalar(pyn[m][:, lsl], sx[:], float(-H / 2.0),
                                        float(0.5 - H / 2.0), ALU.mult, ALU.add)
                nc.vector.tensor_tensor(aef[m][:, lsl], attr[:, :24, l], qmask[:], ALU.mult)

        # ---- main build + matmul ----
        accT = [cpool.tile([128, Q], F32, tag=f"accT{k}", name=f"accT{k}") for k in range(2)]
        acc_ps = [[accps.tile([128, 512], F32, tag=f"acc{cc}h{h}", name=f"acc{cc}h{h}") for h in range(2)]
                  for cc in range(2)]

        for g in range(NGRP):
            for l, (H, W) in enumerate(FEATS_HW):
                HW = H * W
                KT = (HW + 127) // 128
                fsb = fpool.tile([128, KT * 256], BF16, tag="fsb")
                for kt in range(KT):
                    ksz = min(128, HW - kt * 128)
                    nc.sync.dma_start(fsb[:ksz, kt * 256:(kt + 1) * 256],
                                      t_F[(g, l)][kt * 128:kt * 128 + ksz, :])
                first_gl = (g == 0 and l == 0)
                last_gl = (g == NGRP - 1 and l == NL - 1)
                for half in range(2):
                    Ats = []
                    for mm in range(4):
                        m = half * 4 + mm
                        A = apool.tile([128, HW], BF16, tag="A")
                        Ats.append(A)
                        base = l * 24 + g * 8
                        dx = xpool.tile([128, GSL * MAXW], F32, tag="dx")
                        dxv = dx[:, :GSL * W].rearrange("p (s w) -> p s w", w=W)
                        nc.vector.tensor_tensor(
                            dxv,
                            iox[:, :W].unsqueeze(1).broadcast_to([128, GSL, W]),
                            pxn[m][:, base:base + GSL].unsqueeze(2).broadcast_to([128, GSL, W]),
                            ALU.add)
                        tx = xpool.tile([128, GSL * MAXW], BF16, tag="tx")
                        nc.scalar.activation(dx[:, :GSL * W], dx[:, :GSL * W], ACTF.Abs)
                        nc.scalar.activation(tx[:, :GSL * W], dx[:, :GSL * W], ACTF.Relu,
                                             bias=1.0, scale=-1.0)
                        dy = xpool.tile([128, GSL * MAXH], F32, tag="dy")
                        dyv = dy[:, :GSL * H].rearrange("p (s h) -> p s h", h=H)
                        nc.vector.tensor_tensor(
                            dyv,
                            ioy[:, :H].unsqueeze(1).broadcast_to([128, GSL, H]),
                            pyn[m][:, base:base + GSL].unsqueeze(2).broadcast_to([128, GSL, H]),
                            ALU.add)
                        ty = xpool.tile([128, GSL * MAXH], BF16, tag="ty")
                        nc.scalar.activation(dy[:, :GSL * H], dy[:, :GSL * H], ACTF.Abs)
                        nc.scalar.activation(ty[:, :GSL * H], dy[:, :GSL * H], ACTF.Relu,
                                             bias=1.0, scale=-1.0)
                        txv = tx[:, :GSL * W].rearrange("p (s w) -> p s w", w=W)
                        tyv = ty[:, :GSL * H].rearrange("p (s h) -> p s h", h=H)
                        if l == 0:
                            # strip form: all tensor operands packed stride-1
                            # (qualifies for the DVE 2x bf16 mode); the tmp
                            # outer-product op is eliminated entirely.
                            tya = xpool.tile([128, GSL * MAXH], F32, tag="tya")
                            for js in range(GSL):
                                aesc = aef[m][:, l * 24 + g * 8 + js:l * 24 + g * 8 + js + 1]
                                nc.vector.tensor_scalar(
                                    tya[:, js * H:(js + 1) * H],
                                    tyv[:, js], aesc, None, ALU.mult)
                            Av = A[:].rearrange("p (h w) -> p h w", w=W)
                            for y in range(H):
                                for js in range(GSL):
                                    ysc = tya[:, js * H + y:js * H + y + 1]
                                    if js == 0:
                                        nc.vector.tensor_scalar(
                                            Av[:, y], txv[:, js], ysc, None, ALU.mult)
                                    else:
                                        nc.vector.scalar_tensor_tensor(
                                            Av[:, y], txv[:, js], ysc, Av[:, y],
                                            ALU.mult, ALU.add)
                        else:
                            for js in range(GSL):
                                tmp = tpool.tile([128, HW], BF16, tag="tmp", bufs=1)
                                tmpv = tmp[:].rearrange("p (h w) -> p h w", w=W)
                                nc.vector.tensor_tensor(
                                    tmpv,
                                    txv[:, js].unsqueeze(1).broadcast_to([128, H, W]),
                                    tyv[:, js].unsqueeze(2).broadcast_to([128, H, W]),
                                    ALU.mult)
                                aesc = aef[m][:, l * 24 + g * 8 + js:l * 24 + g * 8 + js + 1]
                                if js == 0:
                                    nc.vector.tensor_scalar(A[:], tmp[:], aesc, None, ALU.mult)
                                else:
                                    nc.vector.scalar_tensor_tensor(
                                        A[:], tmp[:], aesc, A[:], ALU.mult, ALU.add)
                    for kt in range(KT):
                        ksz = min(128, HW - kt * 128)
                        AT = atp.tile([128, 512], BF16, tag="AT")
                        for mm in range(4):
                            tp = pspool.tile([128, 128], BF16, tag="tp", bufs=2)
                            nc.tensor.transpose(tp[:ksz, :],
                                                Ats[mm][:, kt * 128:kt * 128 + ksz],
                                                ident[:])
                            nc.scalar.copy(AT[:ksz, mm * 128:(mm + 1) * 128], tp[:ksz, :])
                        for cc in range(2):
                            nc.tensor.matmul(
                                acc_ps[cc][half][:],
                                fsb[:ksz, kt * 256 + cc * 128:kt * 256 + (cc + 1) * 128],
                                AT[:ksz, :],
                                start=(first_gl and kt == 0),
                                stop=(last_gl and kt == KT - 1))

        for cc in range(2):
            for half in range(2):
                nc.vector.tensor_copy(accT[cc][:, half * 512:(half + 1) * 512],
                                      acc_ps[cc][half][:])

        # ---- final linear + bias + residual ----
        qres = [ppool.tile([128, Q], F32, tag=f"qres{k}", name=f"qres{k}", bufs=1) for k in range(2)]
        for k in range(2):
            nc.sync.dma_start(qres[k][:], t_qresT[k * 128:(k + 1) * 128, :])
        for cc in range(2):
            for qc in range(2):
                op = pspool.tile([128, 512], F32, tag="scps", name="outp", bufs=2)
                for k in range(2):
                    nc.tensor.matmul(op[:],
                                     wout[k][:, cc * 128:(cc + 1) * 128],
                                     accT[k][:, qc * 512:(qc + 1) * 512],
                                     start=(k == 0), stop=(k == 1))
                ob = tpool.tile([128, 512], F32, tag="ob")
                nc.vector.scalar_tensor_tensor(
                    ob[:], op[:], boutc[cc][:, 0:1],
                    qres[cc][:, qc * 512:(qc + 1) * 512], ALU.add, ALU.add)
                nc.sync.dma_start(t_out[cc * 128:(cc + 1) * 128, qc * 512:(qc + 1) * 512],
                                  ob[:])
    nc.compile()
    return nc


def _prep_inputs(inp):
    query = np.ascontiguousarray(inp["query"][0], np.float32)
    qpos = np.ascontiguousarray(inp["query_pos"][0], np.float32)
    ref_q = np.transpose(inp["reference_points"], (0, 2, 3, 1, 4)).reshape(Q, NPIL, 3)
    ref_scaled = (ref_q * PC_SPAN + PC_LOW).astype(np.float32)
    L_all = np.asarray(inp["lidar2img"][0], np.float32)
    qT = np.ascontiguousarray(query.T)
    qposT = np.ascontiguousarray(qpos.T)
    iox = np.tile(np.arange(MAXW, dtype=np.float32), (128, 1))
    ioy = np.tile(np.arange(MAXH, dtype=np.float32), (128, 1))
    ident = np.eye(128, dtype=np.float32)
    ones = np.ones((1, Q), np.float32)
    Wout = np.ascontiguousarray(inp["W_out"], np.float32)
    boutC = np.ascontiguousarray(inp["b_out"].reshape(C, 1), np.float32)
    zeros_cq = np.zeros((C, Q), np.float32)

    in_maps = []
    for core in range(8):
        slots = [_slot_decode(core * NSLOT + j) for j in range(NSLOT)]
        W_off_c = np.zeros((C, 256), np.float32)
        b_off_c = np.zeros((1, 256), np.float32)
        W_attn_c = np.zeros((C, 128), np.float32)
        b_attn_c = np.zeros((1, 128), np.float32)
        refc = np.zeros((Q, NSLOT, 4, 3), np.float32)
        Lexp = np.zeros((12, NSLOT, 4), np.float32)
        for j, (n, h, pil, t) in enumerate(slots):
            for l in range(NL):
                for xy in range(2):
                    src = (((h * NL + l) * NPIL + pil) * NPT + t) * 2 + xy
                    dst = (j * NL + l) * 2 + xy
                    W_off_c[:, dst] = inp["W_off"][:, src]
                    b_off_c[0, dst] = inp["b_off"][src]
                srca = (h * PP + pil * NPT + t) * NL + l
                W_attn_c[:, j * NL + l] = inp["W_attn"][:, srca]
                b_attn_c[0, j * NL + l] = inp["b_attn"][srca]
            order = [pil] + [p for p in range(4) if p != pil]
            refc[:, j] = ref_scaled[:, order]
            for i in range(3):
                for k in range(4):
                    Lexp[i * 4 + k, j, :] = L_all[n][i, k]
        m = {
            "qT": qT, "qposT": qposT,
            "qresT": qT if core == 0 else zeros_cq,
            "Woff": W_off_c, "boff": b_off_c,
            "Wattn": W_attn_c, "battn": b_attn_c,
            "Wout": Wout,
            "boutC": boutC if core == 0 else np.zeros((C, 1), np.float32),
            "refS": np.ascontiguousarray(refc.reshape(Q, NSLOT * 12)),
            "Lexp": np.ascontiguousarray(
                np.tile(Lexp.reshape(1, 12 * NSLOT * 4), (128, 1))),
            "iox": iox, "ioy": ioy, "ones": ones,
            "ident": np.eye(128, dtype=np.float32).astype(__import__("ml_dtypes").bfloat16),
        }
        for g in range(NGRP):
            cam = (core * NGRP + g) // 4
            for l, (H, W) in enumerate(FEATS_HW):
                F = np.asarray(inp[f"feat{l}"][0, cam], np.float32).reshape(C, H * W)
                import ml_dtypes
                m[f"F{g}{l}"] = np.ascontiguousarray(F.T).astype(ml_dtypes.bfloat16)
        in_maps.append(m)
    return in_maps


_NC = None


def kernel(**inputs):
    global _NC
    inp = {k: np.asarray(v) for k, v in inputs.items()}
    if _NC is None:
        _NC = _build_program()
    in_maps = _prep_inputs(inp)
    res = run_bass_kernel_spmd(_NC, in_maps, core_ids=list(range(8)))
    out = np.zeros((C, Q), np.float32)
    for r in res.results:
        out += np.asarray(r["outT"], np.float32)
    return np.ascontiguousarray(out.T).reshape(1, Q, C)
